# revision 1
# baseline (speedup 1.0000x reference)
"""Trainium2 Bass kernel for a 2-layer GAT block (gnn_message_passing).

Strategy (8 NeuronCores, dst-node sharding):
  - Host preprocessing: add self-loops, sort edges by dst, shard by dst range
    (6250 nodes/core), group dsts into 128-wide groups. Within each
    (core, group) the edges are split by source half (src < N/2 vs >=) so
    gather indices fit int16, and each half is padded to a multiple of 128;
    block counts are uniformized across cores (max over cores) so the SPMD
    program is identical on every core (only index data differs).
  - Phase A: own-chunk node transform h1 = x @ W1ext (W1ext carries the
    attention projections a_src/a_dst and the residual Wfc as extra columns).
    Rows are written into a 320-float node table: [msg 256 | alsrc 4 |
    alde 4 | scratch/pad 56] (320 keeps the row stride a 256B multiple,
    which gpsimd.dma_gather requires).
  - AllGather the table -> full [N, 320] table in every core's HBM.
  - Edge phase, per 128-dst group: THREE gpsimd.dma_gather calls (Q7 SWDGE
    ucode, one instruction for thousands of descriptors): the group's src
    rows from each table half, and the per-edge a_dst values (64-float tail
    slice, indexed by core-local dst) from the CORE-LOCAL table. Then
    ex = exp(lrelu(alsrc + aldst)) batched per group into the rows' scratch
    column, and per 128-edge block: one-hot S from dstloc (vector engine),
    messages scaled by ex, segment-sum via matmul S.T @ [row | ex]
    accumulated in PSUM over the group's blocks (the ex column sums give
    the softmax denominators).
  - Layer 2 identical (plus mean over heads + residual at evacuation).
"""

import numpy as np

import concourse.bass as bass
import concourse.bacc as bacc
import concourse.mybir as mybir
import concourse.tile as tile
from concourse import library_config
from concourse.bass_utils import run_bass_kernel_spmd

# Problem constants (hardcoded per harness contract)
N = 50000
E = 800000
IN_C = 128
OUT_C = 64
HEADS = 4
NEG_SLOPE = 0.2
N_CORES = 8

P = 128            # partitions
TROW = 320         # table row stride (f32); 1280B = 256B multiple
TUSED = 264        # msg 256 | alsrc 4 | alde 4
TEX = 268          # ex scratch at cols 264:268

FP32 = mybir.dt.float32
I16 = mybir.dt.int16

# timing-triage flags (set by triage.py; default off)
SKIP_EDGE = False
SKIP_AG = False
GATHER_ONLY = False


def _ceil_div(a, b):
    return (a + b - 1) // b


def _pack_idxs(flat):
    """flat[k] -> int16 [16, len/16] at (k%16, k//16), for dma_gather."""
    m = len(flat)
    assert m % 16 == 0
    arr = np.zeros((16, m // 16), np.int16)
    arr[np.arange(m) % 16, np.arange(m) // 16] = flat
    return arr


# ---------------------------------------------------------------------------
# Host-side preprocessing
# ---------------------------------------------------------------------------

def _preprocess(edge_index, n, n_cores):
    """Sort/shard/pad edges; build per-core dma_gather index streams with a
    block schedule that is uniform across cores."""
    npc = n // n_cores
    G = _ceil_div(npc, P)
    split = (n + 1) // 2

    src = np.asarray(edge_index[0], dtype=np.int64)
    dst = np.asarray(edge_index[1], dtype=np.int64)
    loops = np.arange(n, dtype=np.int64)
    src = np.concatenate([src, loops]).astype(np.int32)
    dst = np.concatenate([dst, loops]).astype(np.int32)

    order = np.argsort(dst, kind="stable")
    src = src[order]
    dst = dst[order]
    core_bounds = np.searchsorted(dst, np.arange(0, n + 1, npc))

    percore = []
    counts_lo = np.zeros((n_cores, G), dtype=np.int64)
    counts_hi = np.zeros((n_cores, G), dtype=np.int64)
    for m in range(n_cores):
        s0, s1 = core_bounds[m], core_bounds[m + 1]
        cs = src[s0:s1]
        cd = dst[s0:s1] - m * npc
        grp = cd >> 7
        hi = (cs >= split).astype(np.int64)
        o = np.lexsort((cs, hi, grp))
        cs, cd, grp, hi = cs[o], cd[o], grp[o], hi[o]
        gb = np.searchsorted(grp * 2 + hi, np.arange(2 * G + 2))
        counts_lo[m] = gb[1::2][:G] - gb[0::2][:G]
        counts_hi[m] = gb[2::2][:G] - gb[1::2][:G]
        percore.append((cs, cd, gb))

    BGlo = np.maximum(_ceil_div(counts_lo, P).max(axis=0), 1)
    BGhi = np.maximum(_ceil_div(counts_hi, P).max(axis=0), 1)
    BG = (BGlo + BGhi).astype(int)
    B_total = int(BG.sum())

    cores = []
    for m in range(n_cores):
        cs, cd, gb = percore[m]
        dstloc = np.full((P, B_total), -1.0, dtype=np.float32)
        ids_parts = []
        t = 0
        for g in range(G):
            e_src_parts = []
            e_dl_parts = []
            for h, bgh in ((0, int(BGlo[g])), (1, int(BGhi[g]))):
                a, b = gb[2 * g + h], gb[2 * g + h + 1]
                ne = b - a
                npad = bgh * P - ne
                assert npad >= 0
                base = split * h
                e_src_parts.append(np.concatenate(
                    [cs[a:b] - base, np.zeros(npad, np.int32)]))
                e_dl_parts.append(np.concatenate(
                    [(cd[a:b] - g * P).astype(np.float32),
                     np.full(npad, -1.0, np.float32)]
                ))
                # gather index stream for this half (block-major flat order)
                ids_parts.append(_pack_idxs(e_src_parts[-1].astype(np.int16)))
            e_src = np.concatenate(e_src_parts)
            e_dl = np.concatenate(e_dl_parts)
            e_dg = np.concatenate([
                np.concatenate([cd[gb[2 * g]:gb[2 * g + 1]],
                                np.zeros(int(BGlo[g]) * P -
                                         (gb[2 * g + 1] - gb[2 * g]), np.int32)]),
                np.concatenate([cd[gb[2 * g + 1]:gb[2 * g + 2]],
                                np.zeros(int(BGhi[g]) * P -
                                         (gb[2 * g + 2] - gb[2 * g + 1]), np.int32)]),
            ]).astype(np.int16)
            ids_parts.append(_pack_idxs(e_dg))
            for k in range(int(BG[g])):
                dstloc[:, t] = e_dl[k * P : (k + 1) * P]
                t += 1
        assert t == B_total
        ids16 = np.tile(np.concatenate(ids_parts, axis=1), (8, 1))
        cores.append(dict(ids16=ids16, dstloc=dstloc))

    # compile-time column offsets into ids16 (identical for all cores)
    ids_off = []
    c = 0
    for g in range(G):
        lo_off, c = c, c + int(BGlo[g]) * 8
        hi_off, c = c, c + int(BGhi[g]) * 8
        ad_off, c = c, c + int(BG[g]) * 8
        ids_off.append((lo_off, hi_off, ad_off))

    sched = dict(G=G, npc=npc, split=split, BG=BG, BGlo=BGlo.astype(int),
                 BGhi=BGhi.astype(int), B_total=B_total, MAXB=int(BG.max()),
                 ids_cols=c, ids_off=ids_off)
    return sched, cores


# ---------------------------------------------------------------------------
# Device program
# ---------------------------------------------------------------------------

def _build_program(sched, n, in_c, out_c, heads, add_b1, reps=1):
    G = sched["G"]
    npc = sched["npc"]
    split = sched["split"]
    BG, BGlo, BGhi = sched["BG"], sched["BGlo"], sched["BGhi"]
    B_total = sched["B_total"]
    MAXB = sched["MAXB"]
    ids_cols = sched["ids_cols"]
    ids_off = sched["ids_off"]
    HC = heads * out_c                     # 256
    W1COLS = TUSED + out_c                 # 328: W1 | a_src | a_dst | Wfc
    W2COLS = TUSED                         # 264: W2 | a_src | a_dst

    nc = bacc.Bacc(
        "TRN2",
        target_bir_lowering=False,
        debug=False,
        enable_asserts=False,
        num_devices=N_CORES,
        num_swdge_queues=4,
    )

    # ---- I/O ----
    xT = nc.dram_tensor("xT", [in_c, G * P], FP32, kind="ExternalInput")
    ids_d = nc.dram_tensor("ids16", [P, ids_cols], I16, kind="ExternalInput")
    dstloc_d = nc.dram_tensor("dstloc", [P, B_total], FP32, kind="ExternalInput")
    w1ext_d = nc.dram_tensor("w1ext", [in_c, W1COLS], FP32, kind="ExternalInput")
    w2ext_d = nc.dram_tensor("w2ext", [HC, W2COLS], FP32, kind="ExternalInput")
    iota_d = nc.dram_tensor("iota", [P, P], FP32, kind="ExternalInput")
    ident_d = nc.dram_tensor("ident", [P, P], FP32, kind="ExternalInput")
    if add_b1:
        b1rep_d = nc.dram_tensor("b1rep", [P, HC], FP32, kind="ExternalInput")
    out_d = nc.dram_tensor("out", [G * P, out_c], FP32, kind="ExternalOutput")

    with tile.TileContext(nc) as tc:
        with (
            tc.tile_pool(name="const", bufs=1) as cpool,
            tc.tile_pool(name="dram", bufs=1, space="DRAM") as dpool,
        ):
            nc.gpsimd.load_library(library_config.mlp)
            iota_t = cpool.tile([P, P], FP32)
            nc.sync.dma_start(out=iota_t[:], in_=iota_d[:])
            ident_t = cpool.tile([P, P], FP32)
            nc.sync.dma_start(out=ident_t[:], in_=ident_d[:])
            w1_t = cpool.tile([in_c, W1COLS], FP32)
            nc.sync.dma_start(out=w1_t[:], in_=w1ext_d[:])
            w2a_t = cpool.tile([P, W2COLS], FP32)
            nc.sync.dma_start(out=w2a_t[:], in_=w2ext_d[0:P, :])
            w2b_t = cpool.tile([P, W2COLS], FP32)
            nc.sync.dma_start(out=w2b_t[:], in_=w2ext_d[P : 2 * P, :])
            if add_b1:
                b1_t = cpool.tile([P, HC], FP32)
                nc.sync.dma_start(out=b1_t[:], in_=b1rep_d[:])

            ids_t = cpool.tile([P, ids_cols], I16)
            nc.sync.dma_start(out=ids_t[:], in_=ids_d[:])
            dstloc_t = cpool.tile([P, B_total], FP32)
            nc.sync.dma_start(out=dstloc_t[:], in_=dstloc_d[:])

            f1_sb = cpool.tile([P, G * HC], FP32)
            xch_sb = cpool.tile([P, G * out_c], FP32)

            for _rep in range(reps):
              table1_own = dpool.tile([npc, TROW], FP32, tag=f"t1o{_rep}",
                                      name=f"table1_own{_rep}")
              table1 = dpool.tile([n, TROW], FP32, addr_space="Shared",
                                  tag=f"t1{_rep}", name=f"table1{_rep}")
              table2_own = dpool.tile([npc, TROW], FP32, tag=f"t2o{_rep}",
                                      name=f"table2_own{_rep}")
              table2 = dpool.tile([n, TROW], FP32, addr_space="Shared",
                                  tag=f"t2{_rep}", name=f"table2{_rep}")
              # ---------------- Phase A: layer-1 node transform ----------------
              with (
                  tc.tile_pool(name=f"pa{_rep}", bufs=3) as pa,
                  tc.tile_pool(name=f"pa_ps{_rep}", bufs=2, space="PSUM") as pa_ps,
              ):
                  for g in range(G):
                      xt_t = pa.tile([in_c, P], FP32, tag="xt")
                      nc.sync.dma_start(out=xt_t[:], in_=xT[:, g * P : (g + 1) * P])
                      ph = pa_ps.tile([P, W1COLS], FP32, tag="ph")
                      nc.tensor.matmul(
                          ph[:], lhsT=xt_t[:], rhs=w1_t[:], start=True, stop=True
                      )
                      rows = min(P, npc - g * P)
                      tx = pa.tile([P, TUSED], FP32, tag="tx")
                      nc.scalar.copy(tx[:], ph[:, 0:TUSED])
                      nc.vector.tensor_copy(
                          xch_sb[:, g * out_c : (g + 1) * out_c],
                          ph[:, TUSED:W1COLS],
                      )
                      nc.sync.dma_start(
                          out=table1_own[g * P : g * P + rows, 0:TUSED],
                          in_=tx[:rows, :],
                      )

              if not SKIP_AG:
                  nc.gpsimd.collective_compute(
                      "AllGather",
                      mybir.AluOpType.bypass,
                      replica_groups=[list(range(N_CORES))],
                      ins=[table1_own[:].opt()],
                      outs=[table1[:].opt()],
                  )

              # ---------------- Edge phase ----------------
              def edge_phase(table, table_own, layer):
                  with (
                      tc.tile_pool(name=f"gt{layer}", bufs=2) as gpool,
                      tc.tile_pool(name=f"ad{layer}", bufs=2) as adpool,
                      tc.tile_pool(name=f"sS{layer}", bufs=MAXB + 2) as spool,
                      tc.tile_pool(name=f"ev{layer}", bufs=3) as evpool,
                      tc.tile_pool(name=f"pso{layer}", bufs=2, space="PSUM") as pso,
                  ):
                      qn = [0]
                      t = 0
                      for g in range(G):
                          nblk = int(BG[g])
                          nlo, nhi = int(BGlo[g]), int(BGhi[g])
                          lo_off, hi_off, ad_off = ids_off[g]
                          # a single dma_gather tops out at 1024 indices
                          # (ucode descriptor-ring bound) -> chunk by 8 blocks
                          CH = 8
                          gt = gpool.tile([P, MAXB * TROW], FP32, tag="g")
                          gt3 = gt[:].rearrange("p (c e) -> p c e", e=TROW)
                          for c0 in range(0, nlo, CH):
                              cb = min(CH, nlo - c0)
                              nc.gpsimd.dma_gather(
                                  gt3[:, c0 : c0 + cb, :], table[0:split, :],
                                  ids_t[:, lo_off + 8 * c0 : lo_off + 8 * (c0 + cb)],
                                  cb * P, cb * P, TROW,
                                  single_packet=False, queue_num=qn[0] % 4,
                              )
                              qn[0] += 1
                          for c0 in range(0, nhi, CH):
                              cb = min(CH, nhi - c0)
                              nc.gpsimd.dma_gather(
                                  gt3[:, nlo + c0 : nlo + c0 + cb, :],
                                  table[split:n, :],
                                  ids_t[:, hi_off + 8 * c0 : hi_off + 8 * (c0 + cb)],
                                  cb * P, cb * P, TROW,
                                  single_packet=False, queue_num=qn[0] % 4,
                              )
                              qn[0] += 1
                          adg = adpool.tile([P, MAXB * 64], FP32, tag="a")
                          adg3 = adg[:].rearrange("p (c e) -> p c e", e=64)
                          for c0 in range(0, nblk, CH):
                              cb = min(CH, nblk - c0)
                              nc.gpsimd.dma_gather(
                                  adg3[:, c0 : c0 + cb, :], table_own[:, 256:TROW],
                                  ids_t[:, ad_off + 8 * c0 : ad_off + 8 * (c0 + cb)],
                                  cb * P, cb * P, 64, elem_step=TROW,
                                  single_packet=False, queue_num=qn[0] % 4,
                              )
                              qn[0] += 1

                          if GATHER_ONLY:
                              t += nblk
                              continue

                          # ex = exp(lrelu(alsrc + aldst)), batched per group,
                          # into the rows' scratch columns (264:268)
                          exv = bass.AP(
                              gt3.tensor,
                              gt3.offset + TUSED,
                              [gt3.ap[0], [TROW, nblk], [1, heads]],
                          )
                          alsrc = bass.AP(
                              gt3.tensor,
                              gt3.offset + HC,
                              [gt3.ap[0], [TROW, nblk], [1, heads]],
                          )
                          nc.vector.tensor_tensor(
                              out=exv,
                              in0=alsrc,
                              in1=adg3[:, 0:nblk, 4 : 4 + heads],
                              op=mybir.AluOpType.add,
                          )
                          # lrelu(z) = max(z, 0.2*z) computed manually (the HW
                          # Lrelu table has a fixed slope, ignoring alpha)
                          lrt = evpool.tile([P, MAXB * heads], FP32, tag="lrt")
                          lrt3 = lrt[:].rearrange("p (c e) -> p c e", e=heads)[
                              :, 0:nblk, :
                          ]
                          nc.vector.tensor_scalar(
                              lrt3, exv, NEG_SLOPE, None, mybir.AluOpType.mult
                          )
                          nc.vector.tensor_tensor(
                              out=exv, in0=exv, in1=lrt3, op=mybir.AluOpType.max
                          )
                          nc.scalar.activation(
                              exv, exv, mybir.ActivationFunctionType.Exp
                          )

                          ps_out = pso.tile([P, TEX], FP32, tag="po")
                          for i in range(nblk):
                              S = spool.tile([P, P], FP32, tag="S")
                              nc.vector.tensor_scalar(
                                  S[:],
                                  iota_t[:],
                                  dstloc_t[:, t + i : t + i + 1],
                                  None,
                                  mybir.AluOpType.is_equal,
                              )
                              msg = gt3[:, i, 0:HC]
                              exs = bass.AP(
                                  gt3.tensor,
                                  gt3.offset + (i * TROW + TUSED),
                                  [gt3.ap[0], [1, heads], [0, out_c]],
                              )
                              nc.vector.tensor_tensor(
                                  out=msg, in0=msg, in1=exs,
                                  op=mybir.AluOpType.mult,
                              )
                              nc.tensor.matmul(
                                  ps_out[:],
                                  lhsT=S[:],
                                  rhs=gt3[:, i, 0:TEX],
                                  start=(i == 0),
                                  stop=(i == nblk - 1),
                              )
                          t += nblk

                          # ---- evacuate group ----
                          rec = evpool.tile([P, heads], FP32, tag="rec")
                          if str(layer).endswith("_1"):
                              nc.vector.tensor_scalar(
                                  rec[:], ps_out[:, TUSED:TEX],
                                  1e-16, None, mybir.AluOpType.add,
                              )
                              nc.vector.reciprocal(rec[:], rec[:])
                              recb = bass.AP(
                                  rec[:].tensor, rec[:].offset,
                                  [rec[:].ap[0], [1, heads], [0, out_c]],
                              )
                              nc.vector.tensor_tensor(
                                  out=f1_sb[:, g * HC : (g + 1) * HC],
                                  in0=ps_out[:, 0:HC],
                                  in1=recb,
                                  op=mybir.AluOpType.mult,
                              )
                              if add_b1:
                                  nc.vector.tensor_tensor(
                                      out=f1_sb[:, g * HC : (g + 1) * HC],
                                      in0=f1_sb[:, g * HC : (g + 1) * HC],
                                      in1=b1_t[:],
                                      op=mybir.AluOpType.add,
                                  )
                          else:
                              nc.vector.tensor_scalar(
                                  rec[:], ps_out[:, TUSED:TEX],
                                  1e-16, float(heads),
                                  mybir.AluOpType.add, mybir.AluOpType.mult,
                              )
                              nc.vector.reciprocal(rec[:], rec[:])
                              recb = bass.AP(
                                  rec[:].tensor, rec[:].offset,
                                  [rec[:].ap[0], [1, heads], [0, out_c]],
                              )
                              tmp = evpool.tile([P, HC], FP32, tag="tmp")
                              nc.vector.tensor_tensor(
                                  out=tmp[:], in0=ps_out[:, 0:HC], in1=recb,
                                  op=mybir.AluOpType.mult,
                              )
                              hsum = evpool.tile([P, out_c], FP32, tag="hsum")
                              tmpv = bass.AP(
                                  tmp[:].tensor, tmp[:].offset,
                                  [tmp[:].ap[0], [1, out_c], [out_c, heads]],
                              )
                              nc.vector.tensor_reduce(
                                  out=hsum[:], in_=tmpv,
                                  axis=mybir.AxisListType.X,
                                  op=mybir.AluOpType.add,
                              )
                              ob = evpool.tile([P, out_c], FP32, tag="ob")
                              nc.vector.tensor_tensor(
                                  out=ob[:], in0=hsum[:],
                                  in1=xch_sb[:, g * out_c : (g + 1) * out_c],
                                  op=mybir.AluOpType.add,
                              )
                              nc.sync.dma_start(
                                  out=out_d[g * P : (g + 1) * P, :], in_=ob[:]
                              )

              if SKIP_EDGE or GATHER_ONLY:
                  nc.vector.memset(f1_sb[:], 0.0)
              if SKIP_EDGE:
                  pass
              else:
                  edge_phase(table1, table1_own, layer=f"{_rep}_1")

              # ---------------- Phase D: layer-2 node transform ----------------
              with (
                  tc.tile_pool(name=f"pd{_rep}", bufs=3) as pd,
                  tc.tile_pool(name=f"pd_ps{_rep}", bufs=2, space="PSUM") as pd_ps,
                  tc.tile_pool(name=f"pd_pt{_rep}", bufs=2, space="PSUM") as pd_pt,
              ):
                  for g in range(G):
                      ph = pd_ps.tile([P, W2COLS], FP32, tag="ph2")
                      for k in range(2):
                          pft = pd_pt.tile([P, P], FP32, tag="pft")
                          nc.tensor.transpose(
                              pft[:],
                              f1_sb[:, g * HC + k * P : g * HC + (k + 1) * P],
                              ident_t[:],
                          )
                          fT = pd.tile([P, P], FP32, tag="fT")
                          nc.scalar.copy(fT[:], pft[:])
                          nc.tensor.matmul(
                              ph[:],
                              lhsT=fT[:],
                              rhs=(w2a_t if k == 0 else w2b_t)[:],
                              start=(k == 0),
                              stop=(k == 1),
                          )
                      rows = min(P, npc - g * P)
                      tx = pd.tile([P, TUSED], FP32, tag="tx2")
                      nc.scalar.copy(tx[:], ph[:, 0:TUSED])
                      nc.sync.dma_start(
                          out=table2_own[g * P : g * P + rows, 0:TUSED],
                          in_=tx[:rows, :],
                      )

              if not SKIP_AG:
                  nc.gpsimd.collective_compute(
                      "AllGather",
                      mybir.AluOpType.bypass,
                      replica_groups=[list(range(N_CORES))],
                      ins=[table2_own[:].opt()],
                      outs=[table2[:].opt()],
                  )

              if not SKIP_EDGE:
                  edge_phase(table2, table2_own, layer=f"{_rep}_2")
              else:
                  ob0 = cpool.tile([P, out_c], FP32)
                  nc.vector.memset(ob0[:], 0.0)
                  for g in range(G):
                      nc.sync.dma_start(out=out_d[g * P : (g + 1) * P, :], in_=ob0[:])

    nc.compile()
    return nc


# ---------------------------------------------------------------------------
# Entry point
# ---------------------------------------------------------------------------

def _build_weight_ext(W1, a_src1, a_dst1, W2, a_src2, a_dst2, Wfc,
                      in_c, out_c, heads):
    hc = heads * out_c
    w1r = W1.reshape(in_c, heads, out_c)
    w1_as = np.einsum("khc,hc->kh", w1r, a_src1)
    w1_ad = np.einsum("khc,hc->kh", w1r, a_dst1)
    w1ext = np.concatenate([W1, w1_as, w1_ad, Wfc], axis=1).astype(np.float32)
    w2r = W2.reshape(hc, heads, out_c)
    w2_as = np.einsum("khc,hc->kh", w2r, a_src2)
    w2_ad = np.einsum("khc,hc->kh", w2r, a_dst2)
    w2ext = np.concatenate([W2, w2_as, w2_ad], axis=1).astype(np.float32)
    return w1ext, w2ext


def _build_in_maps(x, sched, cores, w1ext, w2ext, b1, add_b1):
    G, npc = sched["G"], sched["npc"]
    n, in_c = x.shape
    iota = np.broadcast_to(np.arange(P, dtype=np.float32), (P, P)).copy()
    ident = np.eye(P, dtype=np.float32)
    in_maps = []
    for m in range(N_CORES):
        xpad = np.zeros((G * P, in_c), dtype=np.float32)
        xpad[:npc] = x[m * npc : (m + 1) * npc]
        im = dict(
            xT=np.ascontiguousarray(xpad.T),
            ids16=cores[m]["ids16"],
            dstloc=cores[m]["dstloc"],
            w1ext=w1ext,
            w2ext=w2ext,
            iota=iota,
            ident=ident,
        )
        if add_b1:
            im["b1rep"] = np.broadcast_to(b1, (P, b1.shape[0])).copy()
        in_maps.append(im)
    return in_maps


def kernel(x, edge_index, W1, a_src1, a_dst1, b1, W2, a_src2, a_dst2, b2,
           Wfc, bfc):
    x = np.asarray(x, dtype=np.float32)
    W1 = np.asarray(W1, dtype=np.float32)
    W2 = np.asarray(W2, dtype=np.float32)
    a_src1 = np.asarray(a_src1, dtype=np.float32)
    a_dst1 = np.asarray(a_dst1, dtype=np.float32)
    a_src2 = np.asarray(a_src2, dtype=np.float32)
    a_dst2 = np.asarray(a_dst2, dtype=np.float32)
    Wfc = np.asarray(Wfc, dtype=np.float32)
    b1 = np.asarray(b1, dtype=np.float32)
    b2 = np.asarray(b2, dtype=np.float32)
    bfc = np.asarray(bfc, dtype=np.float32)

    n, in_c = x.shape
    heads, out_c = a_src1.shape
    add_b1 = bool(np.any(b1 != 0))

    sched, cores = _preprocess(edge_index, n, N_CORES)
    npc = sched["npc"]

    w1ext, w2ext = _build_weight_ext(W1, a_src1, a_dst1, W2, a_src2, a_dst2,
                                     Wfc, in_c, out_c, heads)
    nc = _build_program(sched, n, in_c, out_c, heads, add_b1)
    in_maps = _build_in_maps(x, sched, cores, w1ext, w2ext, b1, add_b1)

    res = run_bass_kernel_spmd(nc, in_maps, list(range(N_CORES)))
    global LAST_RESULTS
    LAST_RESULTS = res
    outs = [res.results[m]["out"][:npc] for m in range(N_CORES)]
    out = np.concatenate(outs, axis=0)
    out = out + (b2 + bfc)[None, :].astype(np.float32)
    return out.astype(np.float32)

